# revision 1
# baseline (speedup 1.0000x reference)
"""BiLSTM-CRF kernel for 8 Trainium2 NeuronCores.

Data-parallel over batch (dim 0): 128 rows -> 16 rows/core on cores 0-7.
The emissions projection (h @ h2t_W.T) runs on-device as a Bass/Tile SPMD
kernel; the output is a scalar NLL summed over the batch.
"""

import sys

import numpy as np

sys.path.insert(0, "/opt/trn_rl_repo")

B, S, C = 128, 512, 16
V, CV = 50000, 100
E, CE = 100, 30
H = 128
T = 17
NF = 25
LSTM_IN = E + 3 * NF  # 175
N_CORES = 8
B_LOC = B // N_CORES  # 16
TOK = B_LOC * S  # 8192

LAST_EXEC_NS = None  # set when BASS_TRACE=1 produces a profile

_NC_CACHE = {}


def _sigmoid(x):
    out = np.empty_like(x)
    pos = x >= 0
    out[pos] = 1.0 / (1.0 + np.exp(-x[pos]))
    ex = np.exp(x[~pos])
    out[~pos] = ex / (1.0 + ex)
    return out


def _char_conv_np(ce, W, b):
    # ce: [N, C, CE] (char position, emb dim); W: [O, I, K]
    k = W.shape[2]
    sw = np.lib.stride_tricks.sliding_window_view(ce, k, axis=1)  # [N, P, CE, k]
    n, p = sw.shape[0], sw.shape[1]
    sw = np.ascontiguousarray(sw).reshape(n, p, CE * k)
    Wf = W.reshape(NF, CE * k).astype(np.float32)
    out = sw @ Wf.T + b[None, None, :]  # [N, P, O]
    np.maximum(out, 0.0, out=out)
    return out.max(axis=1)  # [N, O]


def _lstm_dir_np(pre, W_hh, reverse):
    # pre: [B, S, 4H] already has W_ih @ x + b; returns hs [B, S, H]
    n = pre.shape[0]
    h = np.zeros((n, H), np.float32)
    c = np.zeros((n, H), np.float32)
    hs = np.empty((n, S, H), np.float32)
    Wt = np.ascontiguousarray(W_hh.T)
    order = range(S - 1, -1, -1) if reverse else range(S)
    for t in order:
        g = pre[:, t] + h @ Wt
        i = _sigmoid(g[:, :H])
        f = _sigmoid(g[:, H : 2 * H])
        gg = np.tanh(g[:, 2 * H : 3 * H])
        o = _sigmoid(g[:, 3 * H :])
        c = f * c + i * gg
        h = o * np.tanh(c)
        hs[:, t] = h
    return hs


def _logsumexp(a, axis):
    m = a.max(axis=axis, keepdims=True)
    return (m + np.log(np.exp(a - m).sum(axis=axis, keepdims=True))).squeeze(axis)


def _build_emissions_nc():
    """Bass graph: out[17, 8192] = w.T @ h for the core's 16-row shard.

    h passed as two 128-partition chunks (contraction dim 256 = 2x128).
    """
    import concourse.bass as bass
    import concourse.mybir as mybir
    from concourse import tile

    nc = bass.Bass()
    h0 = nc.declare_dram_parameter("h0", [128, TOK], mybir.dt.float32, isOutput=False)
    h1 = nc.declare_dram_parameter("h1", [128, TOK], mybir.dt.float32, isOutput=False)
    w0 = nc.declare_dram_parameter("w0", [128, T], mybir.dt.float32, isOutput=False)
    w1 = nc.declare_dram_parameter("w1", [128, T], mybir.dt.float32, isOutput=False)
    out = nc.declare_dram_parameter("out", [T, TOK], mybir.dt.float32, isOutput=True)

    NT = 512  # moving free dim per matmul
    with tile.TileContext(nc) as tc:
        with (
            tc.tile_pool(name="wp", bufs=1) as wp,
            tc.tile_pool(name="xp", bufs=4) as xp,
            tc.tile_pool(name="pp", bufs=4, space="PSUM") as pp,
            tc.tile_pool(name="op", bufs=4) as op,
        ):
            w0t = wp.tile([128, T], mybir.dt.float32, tag="w0")
            w1t = wp.tile([128, T], mybir.dt.float32, tag="w1")
            nc.sync.dma_start(w0t[:], w0[:])
            nc.sync.dma_start(w1t[:], w1[:])
            for j in range(TOK // NT):
                sl = slice(j * NT, (j + 1) * NT)
                x0 = xp.tile([128, NT], mybir.dt.float32, tag="x0")
                x1 = xp.tile([128, NT], mybir.dt.float32, tag="x1")
                nc.sync.dma_start(x0[:], h0[:, sl])
                nc.sync.dma_start(x1[:], h1[:, sl])
                ps = pp.tile([T, NT], mybir.dt.float32, tag="ps")
                nc.tensor.matmul(ps[:], w0t[:], x0[:], start=True, stop=False)
                nc.tensor.matmul(ps[:], w1t[:], x1[:], start=False, stop=True)
                ot = op.tile([T, NT], mybir.dt.float32, tag="ot")
                nc.vector.tensor_copy(ot[:], ps[:])
                nc.sync.dma_start(out[:, sl], ot[:])
    return nc


def _emissions_device(h):
    """h: [B, S, 2H] f32 -> emissions [B, S, T] via 8-core SPMD matmul."""
    global LAST_EXEC_NS
    from concourse.bass_utils import run_bass_kernel_spmd

    if "nc" not in _NC_CACHE:
        _NC_CACHE["nc"] = _build_emissions_nc()
    nc = _NC_CACHE["nc"]

    w = _NC_CACHE["w"]
    w0 = np.ascontiguousarray(w[:, :128].T)  # [128, 17]
    w1 = np.ascontiguousarray(w[:, 128:].T)
    in_maps = []
    for i in range(N_CORES):
        hc = h[i * B_LOC : (i + 1) * B_LOC].reshape(TOK, 2 * H)
        hT = np.ascontiguousarray(hc.T)  # [256, 8192]
        in_maps.append(
            {
                "h0": np.ascontiguousarray(hT[:128]),
                "h1": np.ascontiguousarray(hT[128:]),
                "w0": w0,
                "w1": w1,
            }
        )
    res = run_bass_kernel_spmd(nc, in_maps, core_ids=list(range(N_CORES)))
    if getattr(res, "exec_time_ns", None):
        LAST_EXEC_NS = res.exec_time_ns
    em = np.empty((B, S, T), np.float32)
    for i in range(N_CORES):
        o = res.results[i]["out"]  # [17, 8192]
        em[i * B_LOC : (i + 1) * B_LOC] = o.T.reshape(B_LOC, S, T)
    return em


def kernel(
    x,
    char_x,
    tags,
    mask,
    word_emb,
    char_emb,
    conv2_W,
    conv2_b,
    conv3_W,
    conv3_b,
    conv4_W,
    conv4_b,
    W_ih_f,
    W_hh_f,
    b_f,
    W_ih_b,
    W_hh_b,
    b_b,
    h2t_W,
    h2t_b,
    crf_start,
    crf_end,
    crf_trans,
):
    xi = np.asarray(x).astype(np.int64)
    cxi = np.asarray(char_x).astype(np.int64)
    tg = np.asarray(tags).astype(np.int64)
    msk = np.asarray(mask).astype(bool)
    word_emb = np.asarray(word_emb, np.float32)
    char_emb = np.asarray(char_emb, np.float32)

    # ---- embeddings + char convs ----
    we = word_emb[xi]  # [B, S, E]
    ce = char_emb[cxi].reshape(B * S, C, CE)  # [BS, C(pos), CE]
    cf = np.concatenate(
        [
            _char_conv_np(ce, np.asarray(conv2_W, np.float32), np.asarray(conv2_b, np.float32)),
            _char_conv_np(ce, np.asarray(conv3_W, np.float32), np.asarray(conv3_b, np.float32)),
            _char_conv_np(ce, np.asarray(conv4_W, np.float32), np.asarray(conv4_b, np.float32)),
        ],
        axis=1,
    ).reshape(B, S, 3 * NF)
    feats = np.concatenate([we, cf], axis=2)  # [B, S, 175]

    # ---- BiLSTM ----
    ff = feats.reshape(B * S, LSTM_IN)
    pre_f = (ff @ np.asarray(W_ih_f, np.float32).T + np.asarray(b_f, np.float32)).reshape(B, S, 4 * H)
    pre_b = (ff @ np.asarray(W_ih_b, np.float32).T + np.asarray(b_b, np.float32)).reshape(B, S, 4 * H)
    h_f = _lstm_dir_np(pre_f, np.asarray(W_hh_f, np.float32), reverse=False)
    h_b = _lstm_dir_np(pre_b, np.asarray(W_hh_b, np.float32), reverse=True)
    h = np.concatenate([h_f, h_b], axis=2)  # [B, S, 256]

    # ---- emissions (on device across 8 NeuronCores) ----
    import os
    import signal

    _NC_CACHE["w"] = np.asarray(h2t_W, np.float32)

    def _alarm(signum, frame):
        raise TimeoutError("device emissions timed out")

    try:
        if os.environ.get("KERNEL_NO_DEVICE"):
            raise RuntimeError("KERNEL_NO_DEVICE set")
        old = None
        try:
            old = signal.signal(signal.SIGALRM, _alarm)
            signal.alarm(int(os.environ.get("KERNEL_DEVICE_TIMEOUT", "420")))
        except ValueError:
            pass  # not in main thread; run unguarded
        try:
            emissions = _emissions_device(h)
        finally:
            if old is not None:
                signal.alarm(0)
                signal.signal(signal.SIGALRM, old)
    except Exception as e:  # noqa: BLE001 - fall back to host on any device failure
        print(f"device emissions failed ({e!r}); falling back to host", file=sys.stderr)
        emissions = h.reshape(B * S, 2 * H) @ _NC_CACHE["w"].T
        emissions = emissions.reshape(B, S, T)
    emissions = emissions + np.asarray(h2t_b, np.float32)

    # ---- CRF NLL ----
    start = np.asarray(crf_start, np.float32)
    end = np.asarray(crf_end, np.float32)
    trans = np.asarray(crf_trans, np.float32)
    maskf = msk.astype(np.float32)

    em_sc = np.take_along_axis(emissions, tg[..., None], axis=2)[..., 0]  # [B,S]
    tr_sc = trans[tg[:, :-1], tg[:, 1:]]  # [B,S-1]
    last_idx = msk.sum(axis=1).astype(np.int64) - 1
    last_tag = tg[np.arange(B), last_idx]
    score = (
        start[tg[:, 0]]
        + em_sc[:, 0]
        + (maskf[:, 1:] * (tr_sc + em_sc[:, 1:])).sum(axis=1)
        + end[last_tag]
    )

    alpha = start[None, :] + emissions[:, 0]  # [B,T]
    for t in range(1, S):
        new = _logsumexp(
            alpha[:, :, None] + trans[None] + emissions[:, t][:, None, :], axis=1
        )
        alpha = np.where(msk[:, t][:, None], new, alpha)
    logZ = _logsumexp(alpha + end[None, :], axis=1)
    return np.float32((logZ - score).sum())



# revision 3
# speedup vs baseline: 1.1308x; 1.1308x over previous
"""BiLSTM-CRF on 8 Trainium2 NeuronCores (Bass/Tile), data-parallel over batch.

Each core owns 16 batch rows; tokens are laid out (s, b). The device computes
the char convs (one-hot matmuls against host-precomputed char_emb @ W tables),
the BiLSTM (all matmul offsets static via block staging), the emissions
projection, and the full CRF (gold score + logZ with colsum renormalisation).
The host does the word-embedding gather, packing, and the final reduction.

A tiny warmup NEFF is dispatched on a background thread at entry: the first
run_bass_kernel_spmd call in a process pays a large one-time init cost that is
graph-size dependent, so we pay it on a 5-instruction kernel concurrently with
host prep + graph build.

Falls back to a pure-numpy host path on any device failure.
"""

import os
import sys
import threading

sys.path.insert(0, "/opt/trn_rl_repo")

import numpy as np

try:
    import ml_dtypes

    BF16 = ml_dtypes.bfloat16
except Exception:  # pragma: no cover
    BF16 = None

B, S, C = 128, 512, 16
V, CV = 50000, 100
E, CE = 100, 30
H = 128
T = 17
NF = 25
LSTM_IN = E + 3 * NF  # 175
N_CORES = 8
B_LOC = B // N_CORES  # 16
N_TOK = B_LOC * S  # 8192
NPOS = N_TOK * C  # 131072
C_SHIFT = float(np.log(T))
N_SHIFTS = 9
SHIFT_K = [2, 2, 3, 3, 3, 4, 4, 4, 4]
SHIFT_OF = [0, 1, 0, 1, 2, 0, 1, 2, 3]
SHIFT_BLK = [0, 0, 1, 1, 1, 2, 2, 2, 2]

LAST_EXEC_NS = None

_STATE = {}


# ---------------------------------------------------------------------------
# BIR post-processing: this walrus build accepts at most ONE sync wait per
# instruction.  Hoist excess waits onto NoOps inserted just before (engine
# execution is in-order, so this is semantics-preserving).
# ---------------------------------------------------------------------------

def _split_waits_json(bir_json: bytes) -> bytes:
    import orjson

    m = orjson.loads(bir_json)
    ctr = 0
    for fn in m["functions"]:
        for b in fn.get("blocks") or []:
            instructions = b.get("instructions")
            if not instructions:
                continue
            out = []
            for ins in instructions:
                si = ins.get("sync_info")
                waits = (si or {}).get("on_wait") or []
                if len(waits) > 1:
                    for w in waits[:-1]:
                        ctr += 1
                        out.append({
                            "engine": ins["engine"],
                            "ins": [],
                            "outs": [],
                            "name": f"{ins['name']}_wsplit{ctr}",
                            "opcode": "NoOp",
                            "sync_info": {"on_update": [], "on_wait": [w]},
                        })
                    si["on_wait"] = waits[-1:]
                out.append(ins)
            b["instructions"] = out
    return orjson.dumps(m)


def _patch_compiler():
    if _STATE.get("patched"):
        return
    from concourse import bass2jax, bass_utils

    orig = bass_utils.compile_bir_kernel

    def patched(bir_json, tmpdir, neff_name="file.neff"):
        return orig(_split_waits_json(bir_json), tmpdir, neff_name)

    bass2jax.compile_bir_kernel = patched
    _STATE["patched"] = True


def _warmup():
    """Dispatch a trivial NEFF to absorb the per-process first-call cost."""
    import concourse.bass as bass
    import concourse.mybir as mybir
    from concourse import tile
    from concourse.bass_utils import run_bass_kernel_spmd

    nc = bass.Bass()
    x_d = nc.declare_dram_parameter("x", [1, 8], mybir.dt.float32, isOutput=False)
    y_d = nc.declare_dram_parameter("y", [1, 8], mybir.dt.float32, isOutput=True)
    with tile.TileContext(nc) as tc:
        with tc.tile_pool(name="s", bufs=1) as sp:
            t = sp.tile([1, 8], mybir.dt.float32, tag="t")
            nc.sync.dma_start(t[:], x_d[:])
            nc.scalar.mul(t[:], t[:], 2.0)
            nc.sync.dma_start(y_d[:], t[:])
    run_bass_kernel_spmd(
        nc, [{"x": np.ones((1, 8), np.float32)}] * N_CORES,
        core_ids=list(range(N_CORES)))


# ---------------------------------------------------------------------------
# Device kernel
# ---------------------------------------------------------------------------

def _build_nc():
    import concourse.bass as bass
    import concourse.mybir as mybir
    from concourse import tile

    f32 = mybir.dt.float32
    bf16 = mybir.dt.bfloat16
    AF = mybir.ActivationFunctionType
    OP = mybir.AluOpType
    AX = mybir.AxisListType
    ds = bass.ds

    nc = bass.Bass()

    dp = lambda n, shp, dt: nc.declare_dram_parameter(n, shp, dt, isOutput=False)
    we_d = dp("we", [E, N_TOK], bf16)
    cids_d = dp("cids", [1, NPOS], bf16)
    tags_d = dp("tags", [1, N_TOK], bf16)
    wih0f_d = dp("wih0f", [128, 4 * H], bf16)
    wih1f_d = dp("wih1f", [48, 4 * H], bf16)
    whhf_d = dp("whhf", [H, 4 * H], bf16)
    wih0b_d = dp("wih0b", [128, 4 * H], bf16)
    wih1b_d = dp("wih1b", [48, 4 * H], bf16)
    whhb_d = dp("whhb", [H, 4 * H], bf16)
    convt_d = dp("convt", [E, N_SHIFTS * NF], bf16)
    convb_d = dp("convb", [96, 1], f32)
    h2tf_d = dp("h2tf", [H, T], bf16)
    h2tb_d = dp("h2tb", [H, T], bf16)
    h2tbias_d = dp("h2tbias", [T, 1], f32)
    expT_d = dp("expT", [T, T], f32)
    trans_d = dp("trans", [T, T], f32)
    startv_d = dp("startv", [T, 1], f32)
    endv_d = dp("endv", [T, 1], f32)
    out_d = nc.declare_dram_parameter("out", [1, 8], f32, isOutput=True)

    with tile.TileContext(nc) as tc:
        with (
            tc.tile_pool(name="persist", bufs=1) as pers,
            tc.tile_pool(name="work", bufs=3) as work,
            tc.tile_pool(name="ids", bufs=3) as idsp,
        ):
            feats0 = pers.tile([128, N_TOK], bf16, tag="feats0")
            feats1 = pers.tile([48, N_TOK], bf16, tag="feats1")
            h_f = pers.tile([H, N_TOK + B_LOC], bf16, tag="h_f")
            h_b = pers.tile([H, N_TOK + B_LOC], bf16, tag="h_b")
            em = pers.tile([T, N_TOK], f32, tag="em")
            ohb = pers.tile([T, N_TOK], f32, tag="ohb")
            iota_i = pers.tile([128, 512], mybir.dt.int32, tag="iota_i")
            iota_f = pers.tile([128, 512], f32, tag="iota_f")
            wih0f = pers.tile([128, 4 * H], bf16, tag="wih0f")
            wih1f = pers.tile([48, 4 * H], bf16, tag="wih1f")
            whhf = pers.tile([H, 4 * H], bf16, tag="whhf")
            wih0b = pers.tile([128, 4 * H], bf16, tag="wih0b")
            wih1b = pers.tile([48, 4 * H], bf16, tag="wih1b")
            whhb = pers.tile([H, 4 * H], bf16, tag="whhb")
            convt = pers.tile([E, N_SHIFTS * NF], bf16, tag="convt")
            convb = pers.tile([96, 1], f32, tag="convb")
            h2tf = pers.tile([H, T], bf16, tag="h2tf")
            h2tb = pers.tile([H, T], bf16, tag="h2tb")
            h2tbias = pers.tile([T, 1], f32, tag="h2tbias")
            expT = pers.tile([T, T], f32, tag="expT")
            transm = pers.tile([T, T], f32, tag="transm")
            startv = pers.tile([T, 1], f32, tag="startv")
            endv = pers.tile([T, 1], f32, tag="endv")
            tagsb = pers.tile([1, N_TOK], bf16, tag="tagsb")
            ones1b = pers.tile([1, 128], bf16, tag="ones1b")
            ones17f = pers.tile([T, 1], f32, tag="ones17f")
            ones117f = pers.tile([1, T], f32, tag="ones117f")
            ones16f = pers.tile([T, 16], f32, tag="ones16f")
            alpha = pers.tile([T, B_LOC], f32, tag="alpha")
            off = pers.tile([1, B_LOC], f32, tag="off")
            cst_f = pers.tile([128, B_LOC], f32, tag="cst_f")
            cst_b = pers.tile([128, B_LOC], f32, tag="cst_b")
            acc_em = pers.tile([T, 16], f32, tag="acc_em")
            acc_tr = pers.tile([T, 16], f32, tag="acc_tr")
            acc_se = pers.tile([T, 2], f32, tag="acc_se")
            out_sb = pers.tile([1, 8], f32, tag="out_sb")
            lzrow = pers.tile([1, B_LOC], f32, tag="lzrow")
            negC = pers.tile([T, 1], f32, tag="negC")

            for dst, src in [
                (wih0f, wih0f_d), (wih1f, wih1f_d), (whhf, whhf_d),
                (wih0b, wih0b_d), (wih1b, wih1b_d), (whhb, whhb_d),
                (convt, convt_d), (convb, convb_d),
                (h2tf, h2tf_d), (h2tb, h2tb_d), (h2tbias, h2tbias_d),
                (expT, expT_d), (transm, trans_d), (startv, startv_d), (endv, endv_d),
                (tagsb, tags_d),
            ]:
                nc.sync.dma_start(dst[:], src[:])
            nc.sync.dma_start(feats0[0:E, :], we_d[:])

            nc.gpsimd.iota(iota_i[:], pattern=[[0, 512]], base=0, channel_multiplier=1)
            nc.vector.tensor_copy(iota_f[:], iota_i[:])
            nc.vector.memset(ones1b[:], 1.0)
            nc.vector.memset(ones17f[:], 1.0)
            nc.vector.memset(ones117f[:], 1.0)
            nc.vector.memset(ones16f[:], 1.0)
            nc.vector.memset(feats1[0:1, :], 1.0)
            nc.vector.memset(h_f[:, 0:B_LOC], 0.0)
            nc.vector.memset(h_b[:, N_TOK:N_TOK + B_LOC], 0.0)
            nc.vector.memset(cst_f[:], 0.0)
            nc.vector.memset(cst_b[:], 0.0)
            nc.vector.memset(off[:], 0.0)
            nc.vector.memset(out_sb[:], 0.0)
            nc.vector.memset(negC[:], -C_SHIFT)
            mask3 = pers.tile([96, 512], f32, tag="mask3")
            nc.vector.memset(mask3[:], 1.0)
            m3 = mask3[:].rearrange("p (t c) -> p t c", c=16)
            for blk, k in enumerate((2, 3, 4)):
                nc.gpsimd.affine_select(
                    m3[blk * 32:blk * 32 + NF], m3[blk * 32:blk * 32 + NF],
                    pattern=[[0, 32], [-1, 16]], compare_op=OP.is_ge,
                    fill=0.0, base=16 - k, channel_multiplier=0,
                )

            # ---- phase 1: char convs ----
            CHAR_UNROLL = 4
            with tc.tile_pool(name="ps_char", bufs=2, space="PSUM") as pchar:
                with tc.For_i(0, NPOS // 512 // CHAR_UNROLL) as jo:
                    idst = idsp.tile([1, 512 * CHAR_UNROLL], bf16, tag="idst")
                    nc.sync.dma_start(idst[:], cids_d[0:1, ds(jo * (512 * CHAR_UNROLL), 512 * CHAR_UNROLL)])
                    cfstage = work.tile([96, 32 * CHAR_UNROLL], bf16, tag="cfstage")
                    for ji in range(CHAR_UNROLL):
                        ps_ids = pchar.tile([128, 512], f32, tag="ps_ids")
                        nc.tensor.matmul(ps_ids[:], ones1b[:, 0:128], idst[0:1, ji * 512:(ji + 1) * 512],
                                         start=True, stop=True)
                        oh = work.tile([128, 516], bf16, tag="oh_c")
                        nc.vector.tensor_tensor(oh[:, 0:512], ps_ids[:], iota_f[:], op=OP.is_equal)
                        nc.vector.memset(oh[:, 512:516], 0.0)
                        ps_cv = pchar.tile([96, 512], f32, tag="ps_cv")
                        for si in range(N_SHIFTS):
                            blk = SHIFT_BLK[si]
                            nc.tensor.matmul(
                                ps_cv[blk * 32:blk * 32 + NF, :],
                                convt[0:E, si * NF:(si + 1) * NF],
                                oh[0:E, SHIFT_OF[si]:SHIFT_OF[si] + 512],
                                start=si in (0, 2, 5), stop=si in (1, 4, 8),
                            )
                        cv = work.tile([96, 512], f32, tag="cv")
                        nc.scalar.activation(cv[:], ps_cv[:], AF.Relu, bias=convb[:])
                        nc.vector.tensor_tensor(cv[:], cv[:], mask3[:], op=OP.mult)
                        cv3 = cv[:].rearrange("p (t c) -> p t c", c=16)
                        nc.vector.tensor_reduce(cfstage[:, ji * 32:(ji + 1) * 32], cv3, axis=AX.X, op=OP.max)
                    tok = ds(jo * (32 * CHAR_UNROLL), 32 * CHAR_UNROLL)
                    # repack rows [0:25 | 32:57 | 64:89] -> feats rows 100..174
                    # (SBUF->SBUF DMA: compute engines cannot write unaligned
                    # partition starts, and pool tiles keep Tile deps intact)
                    nc.sync.dma_start(feats0[E:E + NF, tok], cfstage[0:NF, :])
                    nc.sync.dma_start(feats0[E + NF:128, tok], cfstage[32:35, :])
                    nc.sync.dma_start(feats1[1:23, tok], cfstage[35:57, :])
                    nc.sync.dma_start(feats1[23:48, tok], cfstage[64:89, :])

            # ---- phase 2: BiLSTM (block-staged; all matmul offsets static) ----
            LSTM_BLK = 8
            BW = 16 * LSTM_BLK
            hs_f = pers.tile([H, BW + 16], bf16, tag="hs_f")
            hs_b = pers.tile([H, BW + 16], bf16, tag="hs_b")
            nc.vector.memset(hs_f[:, 0:16], 0.0)
            nc.vector.memset(hs_b[:, BW:BW + 16], 0.0)
            with tc.tile_pool(name="ps_lstm", bufs=2, space="PSUM") as plstm:
                with tc.For_i(0, S // LSTM_BLK) as ko:
                    nc.vector.tensor_copy(hs_f[:, 0:16], hs_f[:, BW:BW + 16])
                    nc.vector.tensor_copy(hs_b[:, BW:BW + 16], hs_b[:, 0:16])
                    f0f = work.tile([128, BW], bf16, tag="f0f")
                    f1f = work.tile([48, BW], bf16, tag="f1f")
                    f0b = work.tile([128, BW], bf16, tag="f0b")
                    f1b = work.tile([48, BW], bf16, tag="f1b")
                    nc.vector.tensor_copy(f0f[:], feats0[:, ds(ko * BW, BW)])
                    nc.vector.tensor_copy(f1f[:], feats1[:, ds(ko * BW, BW)])
                    nc.vector.tensor_copy(f0b[:], feats0[:, ds(N_TOK - BW - ko * BW, BW)])
                    nc.vector.tensor_copy(f1b[:], feats1[:, ds(N_TOK - BW - ko * BW, BW)])
                    for j in range(LSTM_BLK):
                        for is_f in (True, False):
                            if is_f:
                                w0, w1, wh, cst = wih0f, wih1f, whhf, cst_f
                                fc = slice(j * 16, (j + 1) * 16)
                                hr = slice(j * 16, (j + 1) * 16)
                                hw = slice((j + 1) * 16, (j + 2) * 16)
                                f0s, f1s, hs = f0f, f1f, hs_f
                            else:
                                w0, w1, wh, cst = wih0b, wih1b, whhb, cst_b
                                fc = slice((LSTM_BLK - 1 - j) * 16, (LSTM_BLK - j) * 16)
                                hr = slice((LSTM_BLK - j) * 16, (LSTM_BLK - j + 1) * 16)
                                hw = slice((LSTM_BLK - 1 - j) * 16, (LSTM_BLK - j) * 16)
                                f0s, f1s, hs = f0b, f1b, hs_b
                            ps_g = plstm.tile([128, 64], f32, tag="ps_gf" if is_f else "ps_gb")
                            for g in range(4):
                                gs = slice(g * 128, (g + 1) * 128)
                                gc = slice(g * 16, (g + 1) * 16)
                                nc.tensor.matmul(ps_g[:, gc], w0[:, gs], f0s[:, fc], start=True, stop=False)
                                nc.tensor.matmul(ps_g[:, gc], w1[:, gs], f1s[:, fc], start=False, stop=False)
                                nc.tensor.matmul(ps_g[:, gc], wh[:, gs], hs[:, hr], start=False, stop=True)
                            sig = work.tile([128, 48], f32, tag="sig_f" if is_f else "sig_b")
                            gg = work.tile([128, 16], f32, tag="gg_f" if is_f else "gg_b")
                            nc.scalar.activation(sig[:], ps_g[:, 0:48], AF.Sigmoid)
                            nc.scalar.activation(gg[:], ps_g[:, 48:64], AF.Tanh)
                            tmp = work.tile([128, 16], f32, tag="tmp_f" if is_f else "tmp_b")
                            nc.vector.tensor_tensor(tmp[:], sig[:, 0:16], gg[:], op=OP.mult)
                            nc.vector.tensor_tensor(cst[:], cst[:], sig[:, 16:32], op=OP.mult)
                            nc.vector.tensor_tensor(cst[:], cst[:], tmp[:], op=OP.add)
                            th = work.tile([128, 16], f32, tag="th_f" if is_f else "th_b")
                            nc.scalar.activation(th[:], cst[:], AF.Tanh)
                            nc.vector.tensor_tensor(hs[:, hw], sig[:, 32:48], th[:], op=OP.mult)
                    nc.gpsimd.tensor_copy(h_f[:, ds(ko * BW + 16, BW)], hs_f[:, 16:BW + 16])
                    nc.gpsimd.tensor_copy(h_b[:, ds(N_TOK - BW - ko * BW, BW)], hs_b[:, 0:BW])

            # ---- phase 3: emissions ----
            with tc.tile_pool(name="ps_em", bufs=2, space="PSUM") as pem:
                for j in range(N_TOK // 512):
                    ps_e = pem.tile([T, 512], f32, tag="ps_e")
                    nc.tensor.matmul(ps_e[:], h2tf[:], h_f[:, j * 512 + 16:(j + 1) * 512 + 16], start=True, stop=False)
                    nc.tensor.matmul(ps_e[:], h2tb[:], h_b[:, j * 512:(j + 1) * 512], start=False, stop=True)
                    nc.scalar.activation(em[:, j * 512:(j + 1) * 512], ps_e[:], AF.Identity, bias=h2tbias[:])

            # ---- phase 4: tag one-hots + gold score ----
            with tc.tile_pool(name="ps_sc", bufs=2, space="PSUM") as psc:
                for j in range(16):
                    ps_tg = psc.tile([T, 512], f32, tag="ps_tg")
                    nc.tensor.matmul(ps_tg[:], ones1b[:, 0:T], tagsb[0:1, j * 512:(j + 1) * 512], start=True, stop=True)
                    nc.vector.tensor_tensor(ohb[:, j * 512:(j + 1) * 512], ps_tg[:], iota_f[0:T, :], op=OP.is_equal)
                for j in range(16):
                    junk = work.tile([T, 512], f32, tag="junk")
                    nc.vector.scalar_tensor_tensor(
                        junk[:], em[:, j * 512:(j + 1) * 512], 1.0, ohb[:, j * 512:(j + 1) * 512],
                        op0=OP.mult, op1=OP.mult, accum_out=acc_em[:, j:j + 1])
                for j in range(16):
                    ps_t2 = psc.tile([T, 512], f32, tag="ps_t2")
                    nc.tensor.matmul(ps_t2[:], transm[:], ohb[:, j * 512:(j + 1) * 512], start=True, stop=True)
                    w = 512 if j < 15 else 496
                    junk2 = work.tile([T, 512], f32, tag="junk2")
                    nc.vector.scalar_tensor_tensor(
                        junk2[:, 0:w], ps_t2[:, 0:w], 1.0, ohb[:, j * 512 + 16:j * 512 + 16 + w],
                        op0=OP.mult, op1=OP.mult, accum_out=acc_tr[:, j:j + 1])
                junk3 = work.tile([T, 16], f32, tag="junk3")
                nc.vector.scalar_tensor_tensor(
                    junk3[:], ohb[:, 0:16], startv[:], ones16f[:],
                    op0=OP.mult, op1=OP.mult, accum_out=acc_se[:, 0:1])
                junk4 = work.tile([T, 16], f32, tag="junk4")
                nc.vector.scalar_tensor_tensor(
                    junk4[:], ohb[:, N_TOK - 16:N_TOK], endv[:], ones16f[:],
                    op0=OP.mult, op1=OP.mult, accum_out=acc_se[:, 1:2])
                r1 = work.tile([T, 1], f32, tag="r1")
                nc.vector.tensor_reduce(r1[:], acc_em[:], axis=AX.X, op=OP.add)
                r2 = work.tile([T, 1], f32, tag="r2")
                nc.vector.tensor_reduce(r2[:], acc_tr[:], axis=AX.X, op=OP.add)
                r3 = work.tile([T, 1], f32, tag="r3")
                nc.vector.tensor_reduce(r3[:], acc_se[:], axis=AX.X, op=OP.add)
                nc.vector.tensor_tensor(r1[:], r1[:], r2[:], op=OP.add)
                nc.vector.tensor_tensor(r1[:], r1[:], r3[:], op=OP.add)
                ps_sc1 = psc.tile([1, 1], f32, tag="ps_sc1")
                nc.tensor.matmul(ps_sc1[:], ones17f[:], r1[:], start=True, stop=True)
                nc.vector.tensor_copy(out_sb[0:1, 0:1], ps_sc1[:])

            # ---- phase 5: CRF logZ forward scan ----
            with tc.tile_pool(name="ps_crf", bufs=2, space="PSUM") as pcrf:
                nc.scalar.activation(alpha[:], em[:, 0:B_LOC], AF.Identity, bias=startv[:])

                def crf_step(src_tile, em_slice):
                    exps = work.tile([T, B_LOC], f32, tag="exps")
                    nc.scalar.activation(exps[:], alpha[:], AF.Exp, bias=negC[:])
                    ps_a = pcrf.tile([T, B_LOC], f32, tag="ps_a")
                    nc.tensor.matmul(ps_a[:], expT[:], exps[:], start=True, stop=True)
                    lna = work.tile([T, B_LOC], f32, tag="lna")
                    nc.scalar.activation(lna[:], ps_a[:], AF.Ln)
                    nc.vector.tensor_tensor(alpha[:], lna[:], src_tile[:, em_slice], op=OP.add)

                def renorm():
                    exps = work.tile([T, B_LOC], f32, tag="exps")
                    nc.scalar.activation(exps[:], alpha[:], AF.Exp)
                    ps_s = pcrf.tile([1, B_LOC], f32, tag="ps_s")
                    nc.tensor.matmul(ps_s[:], ones17f[:], exps[:], start=True, stop=True)
                    lns = work.tile([1, B_LOC], f32, tag="lns")
                    nc.scalar.activation(lns[:], ps_s[:], AF.Ln)
                    nc.vector.tensor_tensor(off[:], off[:], lns[:], op=OP.add)
                    ps_m = pcrf.tile([T, B_LOC], f32, tag="ps_m")
                    nc.tensor.matmul(ps_m[:], ones117f[:], lns[:], start=True, stop=True)
                    nc.vector.tensor_tensor(alpha[:], alpha[:], ps_m[:], op=OP.subtract)

                with tc.For_i(0, 31) as ko:
                    em_stage = work.tile([T, 256], f32, tag="em_stage")
                    nc.vector.tensor_copy(em_stage[:], em[:, ds(ko * 256 + 16, 256)])
                    for kj in range(16):
                        crf_step(em_stage, slice(kj * 16, (kj + 1) * 16))
                    renorm()
                for t in range(497, 512):
                    crf_step(em, slice(t * 16, (t + 1) * 16))

                exps2 = work.tile([T, B_LOC], f32, tag="exps2")
                nc.scalar.activation(exps2[:], alpha[:], AF.Exp, bias=endv[:])
                ps_s2 = pcrf.tile([1, B_LOC], f32, tag="ps_s2")
                nc.tensor.matmul(ps_s2[:], ones17f[:], exps2[:], start=True, stop=True)
                lns2 = work.tile([1, B_LOC], f32, tag="lns2")
                nc.scalar.activation(lns2[:], ps_s2[:], AF.Ln)
                nc.vector.tensor_tensor(lzrow[:], lns2[:], off[:], op=OP.add)
                nc.vector.tensor_reduce(out_sb[0:1, 1:2], lzrow[:], axis=AX.X, op=OP.add)

            nc.sync.dma_start(out_d[:], out_sb[:])

    return nc


def _host_prep(inputs):
    xi = np.asarray(inputs["x"]).astype(np.int64)
    cxi = np.asarray(inputs["char_x"]).astype(np.int64)
    tg = np.asarray(inputs["tags"]).astype(np.int64)
    word_emb = np.asarray(inputs["word_emb"], np.float32)
    char_emb = np.asarray(inputs["char_emb"], np.float32)

    def gate_perm(w):
        return np.concatenate([w[0:H], w[H:2 * H], w[3 * H:4 * H], w[2 * H:3 * H]], axis=0)

    def prep_lstm(W_ih, W_hh, b):
        W_ih = gate_perm(np.asarray(W_ih, np.float32))
        W_hh = gate_perm(np.asarray(W_hh, np.float32))
        b = gate_perm(np.asarray(b, np.float32).reshape(4 * H, 1))[:, 0]
        wihT = W_ih.T
        wih0 = np.ascontiguousarray(wihT[0:128]).astype(BF16)
        wih1 = np.ascontiguousarray(
            np.concatenate([b[None, :], wihT[128:175]], axis=0)).astype(BF16)
        whh = np.ascontiguousarray(W_hh.T).astype(BF16)
        return wih0, wih1, whh

    wih0f, wih1f, whhf = prep_lstm(inputs["W_ih_f"], inputs["W_hh_f"], inputs["b_f"])
    wih0b, wih1b, whhb = prep_lstm(inputs["W_ih_b"], inputs["W_hh_b"], inputs["b_b"])

    convt = np.zeros((E, N_SHIFTS * NF), np.float32)
    for si, (k, sh) in enumerate(zip(SHIFT_K, SHIFT_OF)):
        Wk = np.asarray(inputs[f"conv{k}_W"], np.float32)
        convt[:, si * NF:(si + 1) * NF] = char_emb @ Wk[:, :, sh].T
    convt = convt.astype(BF16)
    convb = np.zeros((96, 1), np.float32)
    convb[0:NF, 0] = np.asarray(inputs["conv2_b"], np.float32)
    convb[32:32 + NF, 0] = np.asarray(inputs["conv3_b"], np.float32)
    convb[64:64 + NF, 0] = np.asarray(inputs["conv4_b"], np.float32)

    h2t_W = np.asarray(inputs["h2t_W"], np.float32)
    h2tf = np.ascontiguousarray(h2t_W[:, 0:H].T).astype(BF16)
    h2tb = np.ascontiguousarray(h2t_W[:, H:2 * H].T).astype(BF16)
    h2tbias = np.asarray(inputs["h2t_b"], np.float32).reshape(T, 1)
    trans = np.asarray(inputs["crf_trans"], np.float32)
    expT = np.exp(trans)
    startv = np.asarray(inputs["crf_start"], np.float32).reshape(T, 1)
    endv = np.asarray(inputs["crf_end"], np.float32).reshape(T, 1)

    we_all = word_emb[xi].astype(BF16)  # [B, S, E]

    shared = dict(
        wih0f=wih0f, wih1f=wih1f, whhf=whhf, wih0b=wih0b, wih1b=wih1b, whhb=whhb,
        convt=convt, convb=convb, h2tf=h2tf, h2tb=h2tb, h2tbias=h2tbias,
        expT=expT, trans=trans, startv=startv, endv=endv,
    )
    in_maps = []
    for c in range(N_CORES):
        rows = slice(c * B_LOC, (c + 1) * B_LOC)
        we_c = np.ascontiguousarray(we_all[rows].transpose(2, 1, 0).reshape(E, N_TOK))
        cids_c = np.ascontiguousarray(
            cxi[rows].transpose(1, 0, 2).reshape(1, NPOS)).astype(BF16)
        tags_c = np.ascontiguousarray(tg[rows].T.reshape(1, N_TOK)).astype(BF16)
        m = dict(shared)
        m.update(we=we_c, cids=cids_c, tags=tags_c)
        in_maps.append(m)
    return in_maps


def _run_device(inputs):
    global LAST_EXEC_NS
    _patch_compiler()

    warm = threading.Thread(target=_warmup, daemon=True)
    warm.start()

    from concourse.bass_utils import run_bass_kernel_spmd

    nc = _build_nc()
    in_maps = _host_prep(inputs)
    warm.join(timeout=300)

    res = run_bass_kernel_spmd(nc, in_maps, core_ids=list(range(N_CORES)))
    if getattr(res, "exec_time_ns", None):
        LAST_EXEC_NS = res.exec_time_ns
    total = np.float64(0.0)
    corr = B_LOC * (S - 1) * C_SHIFT
    for c in range(N_CORES):
        o = res.results[c]["out"][0]
        total += (float(o[1]) + corr) - float(o[0])
    return np.float32(total)


# ---------------------------------------------------------------------------
# Host fallback (pure numpy) — used if the device path fails.
# ---------------------------------------------------------------------------

def _sigmoid(x):
    out = np.empty_like(x)
    pos = x >= 0
    out[pos] = 1.0 / (1.0 + np.exp(-x[pos]))
    ex = np.exp(x[~pos])
    out[~pos] = ex / (1.0 + ex)
    return out


def _char_conv_np(ce, W, b):
    k = W.shape[2]
    sw = np.lib.stride_tricks.sliding_window_view(ce, k, axis=1)
    n, p = sw.shape[0], sw.shape[1]
    sw = np.ascontiguousarray(sw).reshape(n, p, CE * k)
    Wf = W.reshape(NF, CE * k).astype(np.float32)
    out = sw @ Wf.T + b[None, None, :]
    np.maximum(out, 0.0, out=out)
    return out.max(axis=1)


def _lstm_dir_np(pre, W_hh, reverse):
    n = pre.shape[0]
    h = np.zeros((n, H), np.float32)
    c = np.zeros((n, H), np.float32)
    hs = np.empty((n, S, H), np.float32)
    Wt = np.ascontiguousarray(W_hh.T)
    order = range(S - 1, -1, -1) if reverse else range(S)
    for t in order:
        g = pre[:, t] + h @ Wt
        i = _sigmoid(g[:, :H])
        f = _sigmoid(g[:, H:2 * H])
        gg = np.tanh(g[:, 2 * H:3 * H])
        o = _sigmoid(g[:, 3 * H:])
        c = f * c + i * gg
        h = o * np.tanh(c)
        hs[:, t] = h
    return hs


def _logsumexp(a, axis):
    m = a.max(axis=axis, keepdims=True)
    return (m + np.log(np.exp(a - m).sum(axis=axis, keepdims=True))).squeeze(axis)


def _run_host(inputs):
    xi = np.asarray(inputs["x"]).astype(np.int64)
    cxi = np.asarray(inputs["char_x"]).astype(np.int64)
    tg = np.asarray(inputs["tags"]).astype(np.int64)
    msk = np.asarray(inputs["mask"]).astype(bool)
    word_emb = np.asarray(inputs["word_emb"], np.float32)
    char_emb = np.asarray(inputs["char_emb"], np.float32)

    we = word_emb[xi]
    ce = char_emb[cxi].reshape(B * S, C, CE)
    cf = np.concatenate(
        [
            _char_conv_np(ce, np.asarray(inputs["conv2_W"], np.float32), np.asarray(inputs["conv2_b"], np.float32)),
            _char_conv_np(ce, np.asarray(inputs["conv3_W"], np.float32), np.asarray(inputs["conv3_b"], np.float32)),
            _char_conv_np(ce, np.asarray(inputs["conv4_W"], np.float32), np.asarray(inputs["conv4_b"], np.float32)),
        ],
        axis=1,
    ).reshape(B, S, 3 * NF)
    feats = np.concatenate([we, cf], axis=2)

    ff = feats.reshape(B * S, LSTM_IN)
    pre_f = (ff @ np.asarray(inputs["W_ih_f"], np.float32).T + np.asarray(inputs["b_f"], np.float32)).reshape(B, S, 4 * H)
    pre_b = (ff @ np.asarray(inputs["W_ih_b"], np.float32).T + np.asarray(inputs["b_b"], np.float32)).reshape(B, S, 4 * H)
    h_f = _lstm_dir_np(pre_f, np.asarray(inputs["W_hh_f"], np.float32), reverse=False)
    h_b = _lstm_dir_np(pre_b, np.asarray(inputs["W_hh_b"], np.float32), reverse=True)
    h = np.concatenate([h_f, h_b], axis=2)

    emissions = (h.reshape(B * S, 2 * H) @ np.asarray(inputs["h2t_W"], np.float32).T).reshape(B, S, T)
    emissions = emissions + np.asarray(inputs["h2t_b"], np.float32)

    start = np.asarray(inputs["crf_start"], np.float32)
    end = np.asarray(inputs["crf_end"], np.float32)
    trans = np.asarray(inputs["crf_trans"], np.float32)
    maskf = msk.astype(np.float32)

    em_sc = np.take_along_axis(emissions, tg[..., None], axis=2)[..., 0]
    tr_sc = trans[tg[:, :-1], tg[:, 1:]]
    last_idx = msk.sum(axis=1).astype(np.int64) - 1
    last_tag = tg[np.arange(B), last_idx]
    score = (
        start[tg[:, 0]]
        + em_sc[:, 0]
        + (maskf[:, 1:] * (tr_sc + em_sc[:, 1:])).sum(axis=1)
        + end[last_tag]
    )

    alpha = start[None, :] + emissions[:, 0]
    for t in range(1, S):
        new = _logsumexp(alpha[:, :, None] + trans[None] + emissions[:, t][:, None, :], axis=1)
        alpha = np.where(msk[:, t][:, None], new, alpha)
    logZ = _logsumexp(alpha + end[None, :], axis=1)
    return np.float32((logZ - score).sum())


def kernel(**inputs):
    msk = np.asarray(inputs["mask"]).astype(bool)
    use_device = (
        BF16 is not None
        and msk.all()
        and not os.environ.get("KERNEL_NO_DEVICE")
        and np.asarray(inputs["x"]).shape == (B, S)
    )
    if use_device:
        try:
            return _run_device(inputs)
        except Exception as e:  # noqa: BLE001
            print(f"device path failed ({e!r}); falling back to host", file=sys.stderr)
    return _run_host(inputs)


# revision 4
# speedup vs baseline: 1.6380x; 1.4485x over previous
"""BiLSTM-CRF on 8 Trainium2 NeuronCores (Bass/Tile), data-parallel over batch.

Each core owns 16 batch rows; tokens are laid out (s, b). The device computes
the char convs (one-hot matmuls against host-precomputed char_emb @ W tables),
the BiLSTM (all matmul offsets static via block staging), the emissions
projection, and the full CRF (gold score + logZ with colsum renormalisation).
The host does the word-embedding gather, packing, and the final reduction.

A tiny warmup NEFF is dispatched on a background thread at entry: the first
run_bass_kernel_spmd call in a process pays a large one-time init cost that is
graph-size dependent, so we pay it on a 5-instruction kernel concurrently with
host prep + graph build.

Falls back to a pure-numpy host path on any device failure.
"""

import os
import sys
import threading

sys.path.insert(0, "/opt/trn_rl_repo")

import numpy as np

try:
    import ml_dtypes

    BF16 = ml_dtypes.bfloat16
except Exception:  # pragma: no cover
    BF16 = None

B, S, C = 128, 512, 16
V, CV = 50000, 100
E, CE = 100, 30
H = 128
T = 17
NF = 25
LSTM_IN = E + 3 * NF  # 175
N_CORES = 8
B_LOC = B // N_CORES  # 16
N_TOK = B_LOC * S  # 8192
NPOS = N_TOK * C  # 131072
C_SHIFT = float(np.log(T))
N_SHIFTS = 9
SHIFT_K = [2, 2, 3, 3, 3, 4, 4, 4, 4]
SHIFT_OF = [0, 1, 0, 1, 2, 0, 1, 2, 3]
SHIFT_BLK = [0, 0, 1, 1, 1, 2, 2, 2, 2]

LAST_EXEC_NS = None

_STATE = {}


# ---------------------------------------------------------------------------
# BIR post-processing: this walrus build accepts at most ONE sync wait per
# instruction.  Hoist excess waits onto NoOps inserted just before (engine
# execution is in-order, so this is semantics-preserving).
# ---------------------------------------------------------------------------

def _split_waits_json(bir_json: bytes) -> bytes:
    import orjson

    m = orjson.loads(bir_json)
    ctr = 0
    for fn in m["functions"]:
        for b in fn.get("blocks") or []:
            instructions = b.get("instructions")
            if not instructions:
                continue
            out = []
            for ins in instructions:
                si = ins.get("sync_info")
                waits = (si or {}).get("on_wait") or []
                if len(waits) > 1:
                    for w in waits[:-1]:
                        ctr += 1
                        out.append({
                            "engine": ins["engine"],
                            "ins": [],
                            "outs": [],
                            "name": f"{ins['name']}_wsplit{ctr}",
                            "opcode": "NoOp",
                            "sync_info": {"on_update": [], "on_wait": [w]},
                        })
                    si["on_wait"] = waits[-1:]
                out.append(ins)
            b["instructions"] = out
    return orjson.dumps(m)


def _patch_compiler():
    if _STATE.get("patched"):
        return
    from concourse import bass2jax, bass_utils

    orig = bass_utils.compile_bir_kernel

    def patched(bir_json, tmpdir, neff_name="file.neff"):
        return orig(_split_waits_json(bir_json), tmpdir, neff_name)

    bass2jax.compile_bir_kernel = patched
    _STATE["patched"] = True


def _warmup():
    """Dispatch a trivial NEFF to absorb the per-process first-call cost."""
    import concourse.bass as bass
    import concourse.mybir as mybir
    from concourse import tile
    from concourse.bass_utils import run_bass_kernel_spmd

    nc = bass.Bass()
    x_d = nc.declare_dram_parameter("x", [1, 8], mybir.dt.float32, isOutput=False)
    y_d = nc.declare_dram_parameter("y", [1, 8], mybir.dt.float32, isOutput=True)
    with tile.TileContext(nc) as tc:
        with tc.tile_pool(name="s", bufs=1) as sp:
            t = sp.tile([1, 8], mybir.dt.float32, tag="t")
            nc.sync.dma_start(t[:], x_d[:])
            nc.scalar.mul(t[:], t[:], 2.0)
            nc.sync.dma_start(y_d[:], t[:])
    run_bass_kernel_spmd(
        nc, [{"x": np.ones((1, 8), np.float32)}] * N_CORES,
        core_ids=list(range(N_CORES)))


# ---------------------------------------------------------------------------
# Device kernel
# ---------------------------------------------------------------------------

def _build_nc():
    import concourse.bass as bass
    import concourse.mybir as mybir
    from concourse import tile

    f32 = mybir.dt.float32
    bf16 = mybir.dt.bfloat16
    AF = mybir.ActivationFunctionType
    OP = mybir.AluOpType
    AX = mybir.AxisListType
    ds = bass.ds

    nc = bass.Bass()

    dp = lambda n, shp, dt: nc.declare_dram_parameter(n, shp, dt, isOutput=False)
    we_d = dp("we", [E, N_TOK], bf16)
    cids_d = dp("cids", [1, NPOS], bf16)
    tags_d = dp("tags", [1, N_TOK], bf16)
    wih0f_d = dp("wih0f", [128, 4 * H], bf16)
    wih1f_d = dp("wih1f", [48, 4 * H], bf16)
    whhf_d = dp("whhf", [H, 4 * H], bf16)
    wih0b_d = dp("wih0b", [128, 4 * H], bf16)
    wih1b_d = dp("wih1b", [48, 4 * H], bf16)
    whhb_d = dp("whhb", [H, 4 * H], bf16)
    convt_d = dp("convt", [E, N_SHIFTS * NF], bf16)
    convb_d = dp("convb", [96, 1], f32)
    h2tf_d = dp("h2tf", [H, T], bf16)
    h2tb_d = dp("h2tb", [H, T], bf16)
    h2tbias_d = dp("h2tbias", [T, 1], f32)
    expT_d = dp("expT", [T, T], f32)
    trans_d = dp("trans", [T, T], f32)
    startv_d = dp("startv", [T, 1], f32)
    endv_d = dp("endv", [T, 1], f32)
    out_d = nc.declare_dram_parameter("out", [1, 8], f32, isOutput=True)

    with tile.TileContext(nc) as tc:
        with (
            tc.tile_pool(name="persist", bufs=1) as pers,
            tc.tile_pool(name="work", bufs=3) as work,
            tc.tile_pool(name="ids", bufs=3) as idsp,
            tc.tile_pool(name="dram", bufs=1, space="DRAM") as dramp,
        ):
            cfdram = dramp.tile([96, N_TOK], bf16, tag="cfdram")
            feats0 = pers.tile([128, N_TOK], bf16, tag="feats0")
            feats1 = pers.tile([48, N_TOK], bf16, tag="feats1")
            h_f = pers.tile([H, N_TOK + B_LOC], bf16, tag="h_f")
            h_b = pers.tile([H, N_TOK + B_LOC], bf16, tag="h_b")
            em = pers.tile([T, N_TOK], f32, tag="em")
            ohb = pers.tile([T, N_TOK], f32, tag="ohb")
            iota_i = pers.tile([128, 512], mybir.dt.int32, tag="iota_i")
            iota_f = pers.tile([128, 512], f32, tag="iota_f")
            wih0f = pers.tile([128, 4 * H], bf16, tag="wih0f")
            wih1f = pers.tile([48, 4 * H], bf16, tag="wih1f")
            whhf = pers.tile([H, 4 * H], bf16, tag="whhf")
            wih0b = pers.tile([128, 4 * H], bf16, tag="wih0b")
            wih1b = pers.tile([48, 4 * H], bf16, tag="wih1b")
            whhb = pers.tile([H, 4 * H], bf16, tag="whhb")
            convt = pers.tile([E, N_SHIFTS * NF], bf16, tag="convt")
            convb = pers.tile([96, 1], f32, tag="convb")
            h2tf = pers.tile([H, T], bf16, tag="h2tf")
            h2tb = pers.tile([H, T], bf16, tag="h2tb")
            h2tbias = pers.tile([T, 1], f32, tag="h2tbias")
            expT = pers.tile([T, T], f32, tag="expT")
            transm = pers.tile([T, T], f32, tag="transm")
            startv = pers.tile([T, 1], f32, tag="startv")
            endv = pers.tile([T, 1], f32, tag="endv")
            tagsb = pers.tile([1, N_TOK], bf16, tag="tagsb")
            ones1b = pers.tile([1, 128], bf16, tag="ones1b")
            ones17f = pers.tile([T, 1], f32, tag="ones17f")
            ones117f = pers.tile([1, T], f32, tag="ones117f")
            ones16f = pers.tile([T, 16], f32, tag="ones16f")
            alpha = pers.tile([T, B_LOC], f32, tag="alpha")
            off = pers.tile([1, B_LOC], f32, tag="off")
            cst_f = pers.tile([128, B_LOC], f32, tag="cst_f")
            cst_b = pers.tile([128, B_LOC], f32, tag="cst_b")
            acc_em = pers.tile([T, 16], f32, tag="acc_em")
            acc_tr = pers.tile([T, 16], f32, tag="acc_tr")
            acc_se = pers.tile([T, 2], f32, tag="acc_se")
            out_sb = pers.tile([1, 8], f32, tag="out_sb")
            lzrow = pers.tile([1, B_LOC], f32, tag="lzrow")
            negC = pers.tile([T, 1], f32, tag="negC")

            for dst, src in [
                (wih0f, wih0f_d), (wih1f, wih1f_d), (whhf, whhf_d),
                (wih0b, wih0b_d), (wih1b, wih1b_d), (whhb, whhb_d),
                (convt, convt_d), (convb, convb_d),
                (h2tf, h2tf_d), (h2tb, h2tb_d), (h2tbias, h2tbias_d),
                (expT, expT_d), (transm, trans_d), (startv, startv_d), (endv, endv_d),
                (tagsb, tags_d),
            ]:
                nc.sync.dma_start(dst[:], src[:])
            nc.sync.dma_start(feats0[0:E, :], we_d[:])

            nc.gpsimd.iota(iota_i[:], pattern=[[0, 512]], base=0, channel_multiplier=1)
            nc.vector.tensor_copy(iota_f[:], iota_i[:])
            nc.vector.memset(ones1b[:], 1.0)
            nc.vector.memset(ones17f[:], 1.0)
            nc.vector.memset(ones117f[:], 1.0)
            nc.vector.memset(ones16f[:], 1.0)
            nc.vector.memset(feats1[0:1, :], 1.0)
            nc.vector.memset(h_f[:, 0:B_LOC], 0.0)
            nc.vector.memset(h_b[:, N_TOK:N_TOK + B_LOC], 0.0)
            nc.vector.memset(cst_f[:], 0.0)
            nc.vector.memset(cst_b[:], 0.0)
            nc.vector.memset(off[:], 0.0)
            nc.vector.memset(out_sb[:], 0.0)
            nc.vector.memset(negC[:], -C_SHIFT)
            mask3 = pers.tile([96, 512], f32, tag="mask3")
            nc.vector.memset(mask3[:], 1.0)
            m3 = mask3[:].rearrange("p (t c) -> p t c", c=16)
            for blk, k in enumerate((2, 3, 4)):
                nc.gpsimd.affine_select(
                    m3[blk * 32:blk * 32 + NF], m3[blk * 32:blk * 32 + NF],
                    pattern=[[0, 32], [-1, 16]], compare_op=OP.is_ge,
                    fill=0.0, base=16 - k, channel_multiplier=0,
                )

            # ---- phase 1: char convs ----
            CHAR_UNROLL = 4
            with tc.tile_pool(name="ps_char", bufs=2, space="PSUM") as pchar:
                with tc.For_i(0, NPOS // 512 // CHAR_UNROLL) as jo:
                    idst = idsp.tile([1, 512 * CHAR_UNROLL], bf16, tag="idst")
                    nc.sync.dma_start(idst[:], cids_d[0:1, ds(jo * (512 * CHAR_UNROLL), 512 * CHAR_UNROLL)])
                    cfstage = work.tile([96, 32 * CHAR_UNROLL], bf16, tag="cfstage")
                    for ji in range(CHAR_UNROLL):
                        ps_ids = pchar.tile([128, 512], f32, tag="ps_ids")
                        nc.tensor.matmul(ps_ids[:], ones1b[:, 0:128], idst[0:1, ji * 512:(ji + 1) * 512],
                                         start=True, stop=True)
                        oh = work.tile([128, 516], bf16, tag="oh_c")
                        nc.vector.tensor_tensor(oh[:, 0:512], ps_ids[:], iota_f[:], op=OP.is_equal)
                        nc.vector.memset(oh[:, 512:516], 0.0)
                        ps_cv = pchar.tile([96, 512], f32, tag="ps_cv")
                        for si in range(N_SHIFTS):
                            blk = SHIFT_BLK[si]
                            nc.tensor.matmul(
                                ps_cv[blk * 32:blk * 32 + NF, :],
                                convt[0:E, si * NF:(si + 1) * NF],
                                oh[0:E, SHIFT_OF[si]:SHIFT_OF[si] + 512],
                                start=si in (0, 2, 5), stop=si in (1, 4, 8),
                            )
                        cv = work.tile([96, 512], f32, tag="cv")
                        nc.scalar.activation(cv[:], ps_cv[:], AF.Relu, bias=convb[:])
                        nc.vector.tensor_tensor(cv[:], cv[:], mask3[:], op=OP.mult)
                        cv3 = cv[:].rearrange("p (t c) -> p t c", c=16)
                        nc.vector.tensor_reduce(cfstage[:, ji * 32:(ji + 1) * 32], cv3, axis=AX.X, op=OP.max)
                    nc.sync.dma_start(cfdram[:, ds(jo * (32 * CHAR_UNROLL), 32 * CHAR_UNROLL)], cfstage[:])
            # repack rows [0:25 | 32:57 | 64:89] -> feats rows 100..174 via a
            # tracked DRAM tile (compute engines cannot write unaligned
            # partition starts; SBUF->SBUF dynamic DMA would need bacc)
            nc.sync.dma_start(feats0[E:E + NF, :], cfdram[0:NF, :])
            nc.sync.dma_start(feats0[E + NF:128, :], cfdram[32:35, :])
            nc.sync.dma_start(feats1[1:23, :], cfdram[35:57, :])
            nc.sync.dma_start(feats1[23:48, :], cfdram[64:89, :])

            # ---- phase 2: BiLSTM (block-staged; all matmul offsets static) ----
            LSTM_BLK = 8
            BW = 16 * LSTM_BLK
            hs_f = pers.tile([H, BW + 16], bf16, tag="hs_f")
            hs_b = pers.tile([H, BW + 16], bf16, tag="hs_b")
            nc.vector.memset(hs_f[:, 0:16], 0.0)
            nc.vector.memset(hs_b[:, BW:BW + 16], 0.0)
            with tc.tile_pool(name="ps_lstm", bufs=2, space="PSUM") as plstm:
                with tc.For_i(0, S // LSTM_BLK) as ko:
                    nc.vector.tensor_copy(hs_f[:, 0:16], hs_f[:, BW:BW + 16])
                    nc.vector.tensor_copy(hs_b[:, BW:BW + 16], hs_b[:, 0:16])
                    f0f = work.tile([128, BW], bf16, tag="f0f")
                    f1f = work.tile([48, BW], bf16, tag="f1f")
                    f0b = work.tile([128, BW], bf16, tag="f0b")
                    f1b = work.tile([48, BW], bf16, tag="f1b")
                    nc.vector.tensor_copy(f0f[:], feats0[:, ds(ko * BW, BW)])
                    nc.vector.tensor_copy(f1f[:], feats1[:, ds(ko * BW, BW)])
                    nc.vector.tensor_copy(f0b[:], feats0[:, ds(N_TOK - BW - ko * BW, BW)])
                    nc.vector.tensor_copy(f1b[:], feats1[:, ds(N_TOK - BW - ko * BW, BW)])
                    for j in range(LSTM_BLK):
                        for is_f in (True, False):
                            if is_f:
                                w0, w1, wh, cst = wih0f, wih1f, whhf, cst_f
                                fc = slice(j * 16, (j + 1) * 16)
                                hr = slice(j * 16, (j + 1) * 16)
                                hw = slice((j + 1) * 16, (j + 2) * 16)
                                f0s, f1s, hs = f0f, f1f, hs_f
                            else:
                                w0, w1, wh, cst = wih0b, wih1b, whhb, cst_b
                                fc = slice((LSTM_BLK - 1 - j) * 16, (LSTM_BLK - j) * 16)
                                hr = slice((LSTM_BLK - j) * 16, (LSTM_BLK - j + 1) * 16)
                                hw = slice((LSTM_BLK - 1 - j) * 16, (LSTM_BLK - j) * 16)
                                f0s, f1s, hs = f0b, f1b, hs_b
                            ps_g = plstm.tile([128, 64], f32, tag="ps_gf" if is_f else "ps_gb")
                            for g in range(4):
                                gs = slice(g * 128, (g + 1) * 128)
                                gc = slice(g * 16, (g + 1) * 16)
                                nc.tensor.matmul(ps_g[:, gc], w0[:, gs], f0s[:, fc], start=True, stop=False)
                                nc.tensor.matmul(ps_g[:, gc], w1[:, gs], f1s[:, fc], start=False, stop=False)
                                nc.tensor.matmul(ps_g[:, gc], wh[:, gs], hs[:, hr], start=False, stop=True)
                            sig = work.tile([128, 48], f32, tag="sig_f" if is_f else "sig_b")
                            gg = work.tile([128, 16], f32, tag="gg_f" if is_f else "gg_b")
                            nc.scalar.activation(sig[:], ps_g[:, 0:48], AF.Sigmoid)
                            nc.scalar.activation(gg[:], ps_g[:, 48:64], AF.Tanh)
                            tmp = work.tile([128, 16], f32, tag="tmp_f" if is_f else "tmp_b")
                            nc.vector.tensor_tensor(tmp[:], sig[:, 0:16], gg[:], op=OP.mult)
                            nc.vector.tensor_tensor(cst[:], cst[:], sig[:, 16:32], op=OP.mult)
                            nc.vector.tensor_tensor(cst[:], cst[:], tmp[:], op=OP.add)
                            th = work.tile([128, 16], f32, tag="th_f" if is_f else "th_b")
                            nc.scalar.activation(th[:], cst[:], AF.Tanh)
                            nc.vector.tensor_tensor(hs[:, hw], sig[:, 32:48], th[:], op=OP.mult)
                    nc.gpsimd.tensor_copy(h_f[:, ds(ko * BW + 16, BW)], hs_f[:, 16:BW + 16])
                    nc.gpsimd.tensor_copy(h_b[:, ds(N_TOK - BW - ko * BW, BW)], hs_b[:, 0:BW])

            # ---- phase 3: emissions ----
            with tc.tile_pool(name="ps_em", bufs=2, space="PSUM") as pem:
                for j in range(N_TOK // 512):
                    ps_e = pem.tile([T, 512], f32, tag="ps_e")
                    nc.tensor.matmul(ps_e[:], h2tf[:], h_f[:, j * 512 + 16:(j + 1) * 512 + 16], start=True, stop=False)
                    nc.tensor.matmul(ps_e[:], h2tb[:], h_b[:, j * 512:(j + 1) * 512], start=False, stop=True)
                    nc.scalar.activation(em[:, j * 512:(j + 1) * 512], ps_e[:], AF.Identity, bias=h2tbias[:])

            # ---- phase 4: tag one-hots + gold score ----
            with tc.tile_pool(name="ps_sc", bufs=2, space="PSUM") as psc:
                for j in range(16):
                    ps_tg = psc.tile([T, 512], f32, tag="ps_tg")
                    nc.tensor.matmul(ps_tg[:], ones1b[:, 0:T], tagsb[0:1, j * 512:(j + 1) * 512], start=True, stop=True)
                    nc.vector.tensor_tensor(ohb[:, j * 512:(j + 1) * 512], ps_tg[:], iota_f[0:T, :], op=OP.is_equal)
                for j in range(16):
                    junk = work.tile([T, 512], f32, tag="junk")
                    nc.vector.scalar_tensor_tensor(
                        junk[:], em[:, j * 512:(j + 1) * 512], 1.0, ohb[:, j * 512:(j + 1) * 512],
                        op0=OP.mult, op1=OP.mult, accum_out=acc_em[:, j:j + 1])
                for j in range(16):
                    ps_t2 = psc.tile([T, 512], f32, tag="ps_t2")
                    nc.tensor.matmul(ps_t2[:], transm[:], ohb[:, j * 512:(j + 1) * 512], start=True, stop=True)
                    w = 512 if j < 15 else 496
                    junk2 = work.tile([T, 512], f32, tag="junk2")
                    nc.vector.scalar_tensor_tensor(
                        junk2[:, 0:w], ps_t2[:, 0:w], 1.0, ohb[:, j * 512 + 16:j * 512 + 16 + w],
                        op0=OP.mult, op1=OP.mult, accum_out=acc_tr[:, j:j + 1])
                junk3 = work.tile([T, 16], f32, tag="junk3")
                nc.vector.scalar_tensor_tensor(
                    junk3[:], ohb[:, 0:16], startv[:], ones16f[:],
                    op0=OP.mult, op1=OP.mult, accum_out=acc_se[:, 0:1])
                junk4 = work.tile([T, 16], f32, tag="junk4")
                nc.vector.scalar_tensor_tensor(
                    junk4[:], ohb[:, N_TOK - 16:N_TOK], endv[:], ones16f[:],
                    op0=OP.mult, op1=OP.mult, accum_out=acc_se[:, 1:2])
                r1 = work.tile([T, 1], f32, tag="r1")
                nc.vector.tensor_reduce(r1[:], acc_em[:], axis=AX.X, op=OP.add)
                r2 = work.tile([T, 1], f32, tag="r2")
                nc.vector.tensor_reduce(r2[:], acc_tr[:], axis=AX.X, op=OP.add)
                r3 = work.tile([T, 1], f32, tag="r3")
                nc.vector.tensor_reduce(r3[:], acc_se[:], axis=AX.X, op=OP.add)
                nc.vector.tensor_tensor(r1[:], r1[:], r2[:], op=OP.add)
                nc.vector.tensor_tensor(r1[:], r1[:], r3[:], op=OP.add)
                ps_sc1 = psc.tile([1, 1], f32, tag="ps_sc1")
                nc.tensor.matmul(ps_sc1[:], ones17f[:], r1[:], start=True, stop=True)
                nc.vector.tensor_copy(out_sb[0:1, 0:1], ps_sc1[:])

            # ---- phase 5: CRF logZ forward scan ----
            with tc.tile_pool(name="ps_crf", bufs=2, space="PSUM") as pcrf:
                nc.scalar.activation(alpha[:], em[:, 0:B_LOC], AF.Identity, bias=startv[:])

                def crf_step(src_tile, em_slice):
                    exps = work.tile([T, B_LOC], f32, tag="exps")
                    nc.scalar.activation(exps[:], alpha[:], AF.Exp, bias=negC[:])
                    ps_a = pcrf.tile([T, B_LOC], f32, tag="ps_a")
                    nc.tensor.matmul(ps_a[:], expT[:], exps[:], start=True, stop=True)
                    lna = work.tile([T, B_LOC], f32, tag="lna")
                    nc.scalar.activation(lna[:], ps_a[:], AF.Ln)
                    nc.vector.tensor_tensor(alpha[:], lna[:], src_tile[:, em_slice], op=OP.add)

                def renorm():
                    exps = work.tile([T, B_LOC], f32, tag="exps")
                    nc.scalar.activation(exps[:], alpha[:], AF.Exp)
                    ps_s = pcrf.tile([1, B_LOC], f32, tag="ps_s")
                    nc.tensor.matmul(ps_s[:], ones17f[:], exps[:], start=True, stop=True)
                    lns = work.tile([1, B_LOC], f32, tag="lns")
                    nc.scalar.activation(lns[:], ps_s[:], AF.Ln)
                    nc.vector.tensor_tensor(off[:], off[:], lns[:], op=OP.add)
                    ps_m = pcrf.tile([T, B_LOC], f32, tag="ps_m")
                    nc.tensor.matmul(ps_m[:], ones117f[:], lns[:], start=True, stop=True)
                    nc.vector.tensor_tensor(alpha[:], alpha[:], ps_m[:], op=OP.subtract)

                with tc.For_i(0, 31) as ko:
                    em_stage = work.tile([T, 256], f32, tag="em_stage")
                    nc.vector.tensor_copy(em_stage[:], em[:, ds(ko * 256 + 16, 256)])
                    for kj in range(16):
                        crf_step(em_stage, slice(kj * 16, (kj + 1) * 16))
                    renorm()
                for t in range(497, 512):
                    crf_step(em, slice(t * 16, (t + 1) * 16))

                exps2 = work.tile([T, B_LOC], f32, tag="exps2")
                nc.scalar.activation(exps2[:], alpha[:], AF.Exp, bias=endv[:])
                ps_s2 = pcrf.tile([1, B_LOC], f32, tag="ps_s2")
                nc.tensor.matmul(ps_s2[:], ones17f[:], exps2[:], start=True, stop=True)
                lns2 = work.tile([1, B_LOC], f32, tag="lns2")
                nc.scalar.activation(lns2[:], ps_s2[:], AF.Ln)
                nc.vector.tensor_tensor(lzrow[:], lns2[:], off[:], op=OP.add)
                nc.vector.tensor_reduce(out_sb[0:1, 1:2], lzrow[:], axis=AX.X, op=OP.add)

            nc.sync.dma_start(out_d[:], out_sb[:])

    return nc


def _host_prep(inputs):
    xi = np.asarray(inputs["x"]).astype(np.int64)
    cxi = np.asarray(inputs["char_x"]).astype(np.int64)
    tg = np.asarray(inputs["tags"]).astype(np.int64)
    word_emb = np.asarray(inputs["word_emb"], np.float32)
    char_emb = np.asarray(inputs["char_emb"], np.float32)

    def gate_perm(w):
        return np.concatenate([w[0:H], w[H:2 * H], w[3 * H:4 * H], w[2 * H:3 * H]], axis=0)

    def prep_lstm(W_ih, W_hh, b):
        W_ih = gate_perm(np.asarray(W_ih, np.float32))
        W_hh = gate_perm(np.asarray(W_hh, np.float32))
        b = gate_perm(np.asarray(b, np.float32).reshape(4 * H, 1))[:, 0]
        wihT = W_ih.T
        wih0 = np.ascontiguousarray(wihT[0:128]).astype(BF16)
        wih1 = np.ascontiguousarray(
            np.concatenate([b[None, :], wihT[128:175]], axis=0)).astype(BF16)
        whh = np.ascontiguousarray(W_hh.T).astype(BF16)
        return wih0, wih1, whh

    wih0f, wih1f, whhf = prep_lstm(inputs["W_ih_f"], inputs["W_hh_f"], inputs["b_f"])
    wih0b, wih1b, whhb = prep_lstm(inputs["W_ih_b"], inputs["W_hh_b"], inputs["b_b"])

    convt = np.zeros((E, N_SHIFTS * NF), np.float32)
    for si, (k, sh) in enumerate(zip(SHIFT_K, SHIFT_OF)):
        Wk = np.asarray(inputs[f"conv{k}_W"], np.float32)
        convt[:, si * NF:(si + 1) * NF] = char_emb @ Wk[:, :, sh].T
    convt = convt.astype(BF16)
    convb = np.zeros((96, 1), np.float32)
    convb[0:NF, 0] = np.asarray(inputs["conv2_b"], np.float32)
    convb[32:32 + NF, 0] = np.asarray(inputs["conv3_b"], np.float32)
    convb[64:64 + NF, 0] = np.asarray(inputs["conv4_b"], np.float32)

    h2t_W = np.asarray(inputs["h2t_W"], np.float32)
    h2tf = np.ascontiguousarray(h2t_W[:, 0:H].T).astype(BF16)
    h2tb = np.ascontiguousarray(h2t_W[:, H:2 * H].T).astype(BF16)
    h2tbias = np.asarray(inputs["h2t_b"], np.float32).reshape(T, 1)
    trans = np.asarray(inputs["crf_trans"], np.float32)
    expT = np.exp(trans)
    startv = np.asarray(inputs["crf_start"], np.float32).reshape(T, 1)
    endv = np.asarray(inputs["crf_end"], np.float32).reshape(T, 1)

    we_all = word_emb[xi].astype(BF16)  # [B, S, E]

    shared = dict(
        wih0f=wih0f, wih1f=wih1f, whhf=whhf, wih0b=wih0b, wih1b=wih1b, whhb=whhb,
        convt=convt, convb=convb, h2tf=h2tf, h2tb=h2tb, h2tbias=h2tbias,
        expT=expT, trans=trans, startv=startv, endv=endv,
    )
    in_maps = []
    for c in range(N_CORES):
        rows = slice(c * B_LOC, (c + 1) * B_LOC)
        we_c = np.ascontiguousarray(we_all[rows].transpose(2, 1, 0).reshape(E, N_TOK))
        cids_c = np.ascontiguousarray(
            cxi[rows].transpose(1, 0, 2).reshape(1, NPOS)).astype(BF16)
        tags_c = np.ascontiguousarray(tg[rows].T.reshape(1, N_TOK)).astype(BF16)
        m = dict(shared)
        m.update(we=we_c, cids=cids_c, tags=tags_c)
        in_maps.append(m)
    return in_maps


def _run_device(inputs):
    global LAST_EXEC_NS
    _patch_compiler()

    warm = threading.Thread(target=_warmup, daemon=True)
    warm.start()

    from concourse.bass_utils import run_bass_kernel_spmd

    nc = _build_nc()
    in_maps = _host_prep(inputs)
    warm.join(timeout=300)

    res = run_bass_kernel_spmd(nc, in_maps, core_ids=list(range(N_CORES)))
    if getattr(res, "exec_time_ns", None):
        LAST_EXEC_NS = res.exec_time_ns
    total = np.float64(0.0)
    corr = B_LOC * (S - 1) * C_SHIFT
    for c in range(N_CORES):
        o = res.results[c]["out"][0]
        total += (float(o[1]) + corr) - float(o[0])
    return np.float32(total)


# ---------------------------------------------------------------------------
# Host fallback (pure numpy) — used if the device path fails.
# ---------------------------------------------------------------------------

def _sigmoid(x):
    out = np.empty_like(x)
    pos = x >= 0
    out[pos] = 1.0 / (1.0 + np.exp(-x[pos]))
    ex = np.exp(x[~pos])
    out[~pos] = ex / (1.0 + ex)
    return out


def _char_conv_np(ce, W, b):
    k = W.shape[2]
    sw = np.lib.stride_tricks.sliding_window_view(ce, k, axis=1)
    n, p = sw.shape[0], sw.shape[1]
    sw = np.ascontiguousarray(sw).reshape(n, p, CE * k)
    Wf = W.reshape(NF, CE * k).astype(np.float32)
    out = sw @ Wf.T + b[None, None, :]
    np.maximum(out, 0.0, out=out)
    return out.max(axis=1)


def _lstm_dir_np(pre, W_hh, reverse):
    n = pre.shape[0]
    h = np.zeros((n, H), np.float32)
    c = np.zeros((n, H), np.float32)
    hs = np.empty((n, S, H), np.float32)
    Wt = np.ascontiguousarray(W_hh.T)
    order = range(S - 1, -1, -1) if reverse else range(S)
    for t in order:
        g = pre[:, t] + h @ Wt
        i = _sigmoid(g[:, :H])
        f = _sigmoid(g[:, H:2 * H])
        gg = np.tanh(g[:, 2 * H:3 * H])
        o = _sigmoid(g[:, 3 * H:])
        c = f * c + i * gg
        h = o * np.tanh(c)
        hs[:, t] = h
    return hs


def _logsumexp(a, axis):
    m = a.max(axis=axis, keepdims=True)
    return (m + np.log(np.exp(a - m).sum(axis=axis, keepdims=True))).squeeze(axis)


def _run_host(inputs):
    xi = np.asarray(inputs["x"]).astype(np.int64)
    cxi = np.asarray(inputs["char_x"]).astype(np.int64)
    tg = np.asarray(inputs["tags"]).astype(np.int64)
    msk = np.asarray(inputs["mask"]).astype(bool)
    word_emb = np.asarray(inputs["word_emb"], np.float32)
    char_emb = np.asarray(inputs["char_emb"], np.float32)

    we = word_emb[xi]
    ce = char_emb[cxi].reshape(B * S, C, CE)
    cf = np.concatenate(
        [
            _char_conv_np(ce, np.asarray(inputs["conv2_W"], np.float32), np.asarray(inputs["conv2_b"], np.float32)),
            _char_conv_np(ce, np.asarray(inputs["conv3_W"], np.float32), np.asarray(inputs["conv3_b"], np.float32)),
            _char_conv_np(ce, np.asarray(inputs["conv4_W"], np.float32), np.asarray(inputs["conv4_b"], np.float32)),
        ],
        axis=1,
    ).reshape(B, S, 3 * NF)
    feats = np.concatenate([we, cf], axis=2)

    ff = feats.reshape(B * S, LSTM_IN)
    pre_f = (ff @ np.asarray(inputs["W_ih_f"], np.float32).T + np.asarray(inputs["b_f"], np.float32)).reshape(B, S, 4 * H)
    pre_b = (ff @ np.asarray(inputs["W_ih_b"], np.float32).T + np.asarray(inputs["b_b"], np.float32)).reshape(B, S, 4 * H)
    h_f = _lstm_dir_np(pre_f, np.asarray(inputs["W_hh_f"], np.float32), reverse=False)
    h_b = _lstm_dir_np(pre_b, np.asarray(inputs["W_hh_b"], np.float32), reverse=True)
    h = np.concatenate([h_f, h_b], axis=2)

    emissions = (h.reshape(B * S, 2 * H) @ np.asarray(inputs["h2t_W"], np.float32).T).reshape(B, S, T)
    emissions = emissions + np.asarray(inputs["h2t_b"], np.float32)

    start = np.asarray(inputs["crf_start"], np.float32)
    end = np.asarray(inputs["crf_end"], np.float32)
    trans = np.asarray(inputs["crf_trans"], np.float32)
    maskf = msk.astype(np.float32)

    em_sc = np.take_along_axis(emissions, tg[..., None], axis=2)[..., 0]
    tr_sc = trans[tg[:, :-1], tg[:, 1:]]
    last_idx = msk.sum(axis=1).astype(np.int64) - 1
    last_tag = tg[np.arange(B), last_idx]
    score = (
        start[tg[:, 0]]
        + em_sc[:, 0]
        + (maskf[:, 1:] * (tr_sc + em_sc[:, 1:])).sum(axis=1)
        + end[last_tag]
    )

    alpha = start[None, :] + emissions[:, 0]
    for t in range(1, S):
        new = _logsumexp(alpha[:, :, None] + trans[None] + emissions[:, t][:, None, :], axis=1)
        alpha = np.where(msk[:, t][:, None], new, alpha)
    logZ = _logsumexp(alpha + end[None, :], axis=1)
    return np.float32((logZ - score).sum())


def kernel(**inputs):
    msk = np.asarray(inputs["mask"]).astype(bool)
    use_device = (
        BF16 is not None
        and msk.all()
        and not os.environ.get("KERNEL_NO_DEVICE")
        and np.asarray(inputs["x"]).shape == (B, S)
    )
    if use_device:
        try:
            return _run_device(inputs)
        except Exception as e:  # noqa: BLE001
            print(f"device path failed ({e!r}); falling back to host", file=sys.stderr)
    return _run_host(inputs)
